# revision 21
# baseline (speedup 1.0000x reference)
"""Multi-head attention (B=2, S=2048, D=1024, H=16, HD=64) on 8 trn2 cores.

Sharding: core c handles batch b = c // 4 and the 4 heads
[4*(c%4), 4*(c%4)+4)  (tensor-parallel split of the Wq/Wk/Wv column dim,
data-parallel over batch).  Each core computes its heads' full SxS
attention locally; no collectives.

All matmul operands are bf16 (accumulation stays fp32 in PSUM): bf16
weights get the compiler's fast-weight-load (4x faster LDWEIGHTS than the
4-byte fp32r path) and the host-side bf16 conversion halves the input DMA.
End-to-end relative error stays ~6e-3 against the fp32 reference (budget
2e-2).

Per-core algorithm:
  1. X (bf16) DMA'd in s-chunks; X^T built in SBUF via PE transposes
     ([d, s] layout, d on partitions).
  2. Q^T, K^T computed as psum = W[k, dout].T-chain over k  -> [dout, s],
     evacuated to bf16 SBUF (DVE/ACT alternate in the prefix phase).
     V computed in natural [s, dout] layout (lhsT = X^T tiles), stored
     with a per-head all-ones 65th column for the fused softmax-sum.
  3. Attention runs per head-PAIR: the two heads of pair j live at
     partition rows 0:64 / 64:128 of QT/KTt block j, so their score
     matmuls (contraction K=HD=64) issue back-to-back into disjoint
     PE row-groups and execute concurrently.
     Per (pair, 512-wide m-chunk, t-tile):
       ps[128, 1024] = [scoresT_h0 | scoresT_h1]     (2 psum banks)
       ex = exp(ps / 8) in ONE [128,1024] ACT instr (bf16 out)
       av_h[65, 512] += [V_h | 1].T @ ex_h           (row 64 = denom)
     then per head: transpose av back via PE (bf16), one reciprocal +
     one broadcast multiply normalize the whole 512-chunk, DMA out fp32.
"""

import os
from contextlib import ExitStack, nullcontext

import ml_dtypes
import numpy as np

import concourse.bacc as bacc
import concourse.mybir as mybir
import concourse.tile as tile
from concourse.bass_utils import run_bass_kernel_spmd
from concourse.masks import make_identity

B, S, D = 2, 2048, 1024
H, HD = 16, 64
NCORES = 8
HPC = H * B // NCORES          # heads per core = 4
HG = HPC * HD                  # per-core projection width = 256
P = 128
KT = D // P                    # 8 contraction tiles
ST = S // P                    # 16 sequence tiles
MC = 512                       # m-chunk width for the attention loop
NMC = S // MC                  # 4
VW = HD + 1                    # V columns per head incl. ones column = 65

F32 = mybir.dt.float32
BF16 = mybir.dt.bfloat16
EXP = mybir.ActivationFunctionType.Exp
MULT = mybir.AluOpType.mult


def build_nc(reps=1):
    nc = bacc.Bacc(
        "TRN2", target_bir_lowering=False, debug=False, num_devices=NCORES
    )
    x = nc.dram_tensor("x", [S, D], BF16, kind="ExternalInput")
    wq = nc.dram_tensor("wq", [D, HG], BF16, kind="ExternalInput")
    wk = nc.dram_tensor("wk", [D, HG], BF16, kind="ExternalInput")
    wv = nc.dram_tensor("wv", [D, HG], BF16, kind="ExternalInput")
    out = nc.dram_tensor("out", [S, HG], F32, kind="ExternalOutput")

    with tile.TileContext(nc) as tc, ExitStack() as ctx:
        big = ctx.enter_context(tc.tile_pool(name="big", bufs=1))
        xst = ctx.enter_context(tc.tile_pool(name="xst", bufs=4))
        expp = ctx.enter_context(tc.tile_pool(name="expp", bufs=6))
        osbp = ctx.enter_context(tc.tile_pool(name="osbp", bufs=4))
        outp = ctx.enter_context(tc.tile_pool(name="outp", bufs=4))
        recp = ctx.enter_context(tc.tile_pool(name="recp", bufs=8))
        # PSUM budget (8 banks): pp_ss 2 slots x [128,1024]f32 = 4 banks
        # (shared tag for scores/proj/V/X^T-transposes), pp_po 2 x
        # [128,512]bf16 = 2 banks (output transposes), pp_av 2 x [65,512] = 2.
        pp_ss = ctx.enter_context(tc.tile_pool(name="pp_ss", bufs=2, space="PSUM"))
        pp_po = ctx.enter_context(tc.tile_pool(name="pp_po", bufs=2, space="PSUM"))
        pp_av = ctx.enter_context(tc.tile_pool(name="pp_av", bufs=2, space="PSUM"))

        rep_ctx = tc.For_i(0, reps, 1) if reps > 1 else nullcontext()
        with rep_ctx:
            ident = big.tile([P, P], F32)
            make_identity(nc, ident[:])
            identb = big.tile([P, P], BF16)
            nc.vector.tensor_copy(identb[:], ident[:])

            # ---- persistent SBUF tensors (all bf16) ----
            XT = big.tile([P, KT * S], BF16)       # X^T: col(kt, s) = kt*S + s
            WQs = big.tile([P, KT * HG], BF16)     # col(kt, j) = kt*HG + j
            WKs = big.tile([P, KT * HG], BF16)
            WVs = big.tile([P, KT * HG], BF16)
            QT = big.tile([P, 2 * S], BF16)        # col(jtile, m) = jtile*S + m
            KTt = big.tile([P, 2 * S], BF16)
            Vn = big.tile([P, ST * HPC * VW], BF16)  # col(st, h, e) = st*HPC*VW + h*VW + e

            # ---- load weights (one DMA per weight matrix), off SP's queue so
            # they never delay the X chunks that gate the first transposes ----
            for eng, Wt, w in (
                (nc.scalar, WQs, wq),
                (nc.gpsimd, WKs, wk),
                (nc.gpsimd, WVs, wv),
            ):
                eng.dma_start(
                    Wt[:].rearrange("p (k n) -> p k n", n=HG),
                    w[:].rearrange("(k p) n -> p k n", p=P),
                )

            # ones columns of Vn (col 64 of each head block)
            ones_ap = Vn[:].rearrange("p (s h e) -> p s h e", h=HPC, e=VW)[:, :, :, HD:VW]
            ones_stage = big.tile([P, ST * HPC], F32)
            nc.vector.memset(ones_stage[:], 1.0)
            nc.vector.tensor_copy(
                ones_ap,
                ones_stage[:].rearrange("p (s h e) -> p s h e", h=HPC, e=1),
            )

            # ---- prefix building blocks (X^T tiles, per-chunk projections).
            # All psum evacuations run on DVE: ACT must do nothing but exp.
            XT3 = XT[:].rearrange("p (k s) -> p k s", s=S)
            Vn4 = Vn[:].rearrange("p (s h e) -> p s h e", h=HPC, e=VW)

            def emit_x_group(sp):
                # DMA one 2-s-tile chunk of X and transpose it into XT
                xs = xst.tile([P, 2 * D], BF16)
                nc.sync.dma_start(
                    xs[:].rearrange("p (t d) -> p t d", d=D),
                    x[sp * 2 * P:(sp + 1) * 2 * P, :].rearrange("(t p) d -> p t d", p=P),
                )
                for tt in range(2):
                    st = sp * 2 + tt
                    for g in range(2):
                        pt = pp_ss.tile([P, 1024], BF16, tag="ps")
                        for j in range(4):
                            kt = g * 4 + j
                            nc.tensor.transpose(
                                pt[:, j * P:(j + 1) * P],
                                xs[:, tt * D + kt * P: tt * D + (kt + 1) * P],
                                identb[:],
                            )
                        dst = XT3[:, g * 4:(g + 1) * 4, st * P:(st + 1) * P]
                        src = pt[:, 0:512].rearrange("p (k s) -> p k s", s=P)
                        nc.vector.tensor_copy(dst, src)

            def emit_proj_qk_nn(Wt, Ot, j, nn):
                pt = pp_ss.tile([P, 1024], F32, tag="ps", name="pt")
                for kt in range(KT):
                    nc.tensor.matmul(
                        pt[:, 0:512],
                        Wt[:, kt * HG + j * P: kt * HG + (j + 1) * P],
                        XT[:, kt * S + nn * 512: kt * S + (nn + 1) * 512],
                        start=(kt == 0),
                        stop=(kt == KT - 1),
                    )
                nc.vector.tensor_copy(
                    Ot[:, j * S + nn * 512: j * S + (nn + 1) * 512],
                    pt[:, 0:512],
                )

            def emit_proj_qk(Wt, Ot, j):
                for nn in range(4):
                    emit_proj_qk_nn(Wt, Ot, j, nn)

            def emit_v_st(st):
                pt = pp_ss.tile([P, 1024], F32, tag="ps", name="pt")
                for kt in range(KT):
                    nc.tensor.matmul(
                        pt[:, 0:HG],
                        XT[:, kt * S + st * P: kt * S + (st + 1) * P],
                        WVs[:, kt * HG:(kt + 1) * HG],
                        start=(kt == 0),
                        stop=(kt == KT - 1),
                    )
                nc.vector.tensor_copy(
                    Vn4[:, st, :, 0:HD],
                    pt[:, 0:HG].rearrange("p (h e) -> p h e", e=HD),
                )

            def mm_scores(j, t, mc):
                # the two heads' score matmuls contract over disjoint
                # 64-row groups of the PE array -> concurrent execution
                qh = QT[:, j * S:(j + 1) * S]
                kh = KTt[:, j * S:(j + 1) * S]
                ps = pp_ss.tile([P, 2 * MC], F32, tag="ps", name="ps")
                nc.tensor.matmul(
                    ps[:, 0:MC],
                    kh[0:64, t * P:(t + 1) * P],
                    qh[0:64, mc * MC:(mc + 1) * MC],
                    start=True,
                    stop=True,
                )
                nc.tensor.matmul(
                    ps[:, MC:2 * MC],
                    kh[64:128, t * P:(t + 1) * P],
                    qh[64:128, mc * MC:(mc + 1) * MC],
                    start=True,
                    stop=True,
                )
                return ps

            class AttnChunk:
                """One (pair, m-chunk): av accumulators live across split
                emission so prefix work can interleave with the t-loop."""

                def __init__(self, j, mc):
                    self.j, self.mc = j, mc
                    self.av01 = [
                        pp_av.tile([VW, MC], F32, tag="av", name="av"),
                        pp_av.tile([VW, MC], F32, tag="av", name="av"),
                    ]
                    # software pipeline: MM_s(t+1) is emitted BEFORE
                    # exp(t)/MM_av(t) so the PE never sits behind the exp.
                    self.ps = mm_scores(j, 0, mc)

                def run(self, t_lo, t_hi):
                    for t in range(t_lo, t_hi):
                        ps_cur = self.ps
                        if t + 1 < ST:
                            self.ps = mm_scores(self.j, t + 1, self.mc)
                        ex = expp.tile([P, 2 * MC], BF16, tag="ex", name="ex")
                        nc.scalar.activation(
                            ex[:], ps_cur[:], EXP, scale=1.0 / np.sqrt(HD)
                        )
                        for hi in range(2):
                            h = 2 * self.j + hi
                            nc.tensor.matmul(
                                self.av01[hi][:],
                                Vn[:, t * HPC * VW + h * VW: t * HPC * VW + (h + 1) * VW],
                                ex[:, hi * MC:(hi + 1) * MC],
                                start=(t == 0),
                                stop=(t == ST - 1),
                            )

                def finish(self):
                    # evacuate, transpose back, normalize, store (one DMA per
                    # (head, m-chunk)). All 4 transposed 128-blocks land in
                    # ONE [128,512] psum tile so a single reciprocal + a
                    # single broadcast multiply normalize the whole chunk.
                    j, mc = self.j, self.mc
                    NMS = MC // P
                    for hi in range(2):
                        h = 2 * j + hi
                        osb = osbp.tile([VW, MC], BF16, tag="osb", name="osb")
                        nc.vector.tensor_copy(osb[:], self.av01[hi][:])
                        ot = outp.tile([P, NMS * HD], F32, tag="ot", name="ot")
                        po = pp_po.tile([P, 512], BF16, tag="po", name="po")
                        for ms in range(NMS):
                            nc.tensor.transpose(
                                po[:, ms * P:ms * P + VW],
                                osb[0:VW, ms * P:(ms + 1) * P],
                                identb[0:VW, 0:VW],
                            )
                        po3 = po[:].rearrange("p (ms c) -> p ms c", c=P)
                        rec = recp.tile([P, NMS], F32, tag="rec", name="rec")
                        nc.vector.reciprocal(rec[:], po3[:, :, HD])
                        nc.vector.tensor_tensor(
                            ot[:].rearrange("p (ms e) -> p ms e", e=HD),
                            po3[:, :, 0:HD],
                            rec[:].unsqueeze(2).broadcast_to([P, NMS, HD]),
                            MULT,
                        )
                        row0 = mc * MC
                        nc.sync.dma_start(
                            out[row0:row0 + MC, h * HD:(h + 1) * HD].rearrange(
                                "(t p) e -> p t e", p=P
                            ),
                            ot[:].rearrange("p (t e) -> p t e", e=HD),
                        )

            def emit_attn_chunk(j, mc):
                c = AttnChunk(j, mc)
                c.run(0, ST)
                c.finish()

            # ---- emission schedule ----
            # Prefix interleave: per nn-group, land the X chunks, then the
            # K/V/Q projections that depend only on them; attention chunk
            # (pair0, mc0) starts its t-loop as soon as the K^T/V tiles for
            # those t exist, so the ACT exp stream (the kernel bottleneck)
            # starts ~15-20us earlier than a phase-ordered emission.
            # nn=0: X s-tiles 0-3, K(0), V(0-3), Q(0)
            emit_x_group(0)
            emit_x_group(1)
            emit_proj_qk_nn(WKs, KTt, 0, 0)
            for st in range(0, 4):
                emit_v_st(st)
            emit_proj_qk_nn(WQs, QT, 0, 0)
            # nn=1: X s-tiles 4-7, K(1), V(4-7), Q(1), then attention t=0..3
            emit_x_group(2)
            emit_x_group(3)
            emit_proj_qk_nn(WKs, KTt, 0, 1)
            for st in range(4, 8):
                emit_v_st(st)
            emit_proj_qk_nn(WQs, QT, 0, 1)
            c0 = AttnChunk(0, 0)
            c0.run(0, 4)
            # nn=2: X s-tiles 8-11, K(2), attention t=4..7, V(8-11), Q(2)
            emit_x_group(4)
            emit_x_group(5)
            emit_proj_qk_nn(WKs, KTt, 0, 2)
            c0.run(4, 8)
            for st in range(8, 12):
                emit_v_st(st)
            emit_proj_qk_nn(WQs, QT, 0, 2)
            # nn=3: X s-tiles 12-15, K(3), attention t=8..11, V(12-15),
            # Q(3), attention t=12..15
            emit_x_group(6)
            emit_x_group(7)
            emit_proj_qk_nn(WKs, KTt, 0, 3)
            c0.run(8, 12)
            for st in range(12, 16):
                emit_v_st(st)
            emit_proj_qk_nn(WQs, QT, 0, 3)
            c0.run(12, ST)
            c0.finish()
            for mc in range(1, NMC):
                emit_attn_chunk(0, mc)
            emit_proj_qk(WKs, KTt, 1)
            emit_proj_qk(WQs, QT, 1)
            for mc in range(NMC):
                emit_attn_chunk(1, mc)

    nc.compile()
    return nc


_NC = None


def _get_nc():
    global _NC
    if _NC is None:
        _NC = build_nc()
    return _NC


def _shard_inputs(inputs, Wq, Wk, Wv):
    bf = ml_dtypes.bfloat16
    inputs = np.ascontiguousarray(np.asarray(inputs, dtype=np.float32)).astype(bf)
    Wq = np.asarray(Wq, dtype=np.float32).astype(bf)
    Wk = np.asarray(Wk, dtype=np.float32).astype(bf)
    Wv = np.asarray(Wv, dtype=np.float32).astype(bf)
    in_maps = []
    for c in range(NCORES):
        b, g = c // (NCORES // B), c % (NCORES // B)
        sl = slice(g * HG, (g + 1) * HG)
        in_maps.append(
            {
                "x": inputs[b],
                "wq": np.ascontiguousarray(Wq[:, sl]),
                "wk": np.ascontiguousarray(Wk[:, sl]),
                "wv": np.ascontiguousarray(Wv[:, sl]),
            }
        )
    return in_maps


def _gather(results):
    out = np.empty((B, S, H * HD), dtype=np.float32)
    for c in range(NCORES):
        b, g = c // (NCORES // B), c % (NCORES // B)
        out[b, :, g * HG:(g + 1) * HG] = results[c]["out"]
    return out


def kernel(inputs, Wq, Wk, Wv):
    nc = _get_nc()
    in_maps = _shard_inputs(inputs, Wq, Wk, Wv)
    res = run_bass_kernel_spmd(nc, in_maps, core_ids=list(range(NCORES)))
    return _gather(res.results)


# revision 23
# speedup vs baseline: 1.3437x; 1.3437x over previous
"""Multi-head attention (B=2, S=2048, D=1024, H=16, HD=64) on 8 trn2 cores.

Sharding: core c handles batch b = c // 4 and the 4 heads
[4*(c%4), 4*(c%4)+4)  (tensor-parallel split of the Wq/Wk/Wv column dim,
data-parallel over batch).  Each core computes its heads' full SxS
attention locally; no collectives.

All matmul operands are bf16 (accumulation stays fp32 in PSUM): bf16
weights get the compiler's fast-weight-load (4x faster LDWEIGHTS than the
4-byte fp32r path) and the host-side bf16 conversion halves the input DMA.
End-to-end relative error stays ~6e-3 against the fp32 reference (budget
2e-2).

Per-core algorithm:
  1. X (bf16) DMA'd in s-chunks; X^T built in SBUF via PE transposes
     ([d, s] layout, d on partitions).
  2. Q^T, K^T computed as psum = W[k, dout].T-chain over k  -> [dout, s],
     evacuated to bf16 SBUF (DVE/ACT alternate in the prefix phase).
     V computed in natural [s, dout] layout (lhsT = X^T tiles), stored
     with a per-head all-ones 65th column for the fused softmax-sum.
  3. Attention runs per head-PAIR: the two heads of pair j live at
     partition rows 0:64 / 64:128 of QT/KTt block j, so their score
     matmuls (contraction K=HD=64) issue back-to-back into disjoint
     PE row-groups and execute concurrently.
     Per (pair, 512-wide m-chunk, t-tile):
       ps[128, 1024] = [scoresT_h0 | scoresT_h1]     (2 psum banks)
       ex = exp(ps / 8) in ONE [128,1024] ACT instr (bf16 out)
       av_h[65, 512] += [V_h | 1].T @ ex_h           (row 64 = denom)
     then per head: transpose av back via PE (bf16), one reciprocal +
     one broadcast multiply normalize the whole 512-chunk, DMA out fp32.
"""

import os
from contextlib import ExitStack, nullcontext

import ml_dtypes
import numpy as np

import concourse.bacc as bacc
import concourse.mybir as mybir
import concourse.tile as tile
from concourse.bass_utils import run_bass_kernel_spmd
from concourse.masks import make_identity

B, S, D = 2, 2048, 1024
H, HD = 16, 64
NCORES = 8
HPC = H * B // NCORES          # heads per core = 4
HG = HPC * HD                  # per-core projection width = 256
P = 128
KT = D // P                    # 8 contraction tiles
ST = S // P                    # 16 sequence tiles
MC = 512                       # m-chunk width for the attention loop
NMC = S // MC                  # 4
VW = HD + 1                    # V columns per head incl. ones column = 65

F32 = mybir.dt.float32
BF16 = mybir.dt.bfloat16
EXP = mybir.ActivationFunctionType.Exp
MULT = mybir.AluOpType.mult


def build_nc(reps=1):
    nc = bacc.Bacc(
        "TRN2", target_bir_lowering=False, debug=False, num_devices=NCORES
    )
    x = nc.dram_tensor("x", [S, D], BF16, kind="ExternalInput")
    wq = nc.dram_tensor("wq", [D, HG], BF16, kind="ExternalInput")
    wk = nc.dram_tensor("wk", [D, HG], BF16, kind="ExternalInput")
    wv = nc.dram_tensor("wv", [D, HG], BF16, kind="ExternalInput")
    out = nc.dram_tensor("out", [S, HG], F32, kind="ExternalOutput")

    with tile.TileContext(nc) as tc, ExitStack() as ctx:
        big = ctx.enter_context(tc.tile_pool(name="big", bufs=1))
        xst = ctx.enter_context(tc.tile_pool(name="xst", bufs=4))
        expp = ctx.enter_context(tc.tile_pool(name="expp", bufs=6))
        osbp = ctx.enter_context(tc.tile_pool(name="osbp", bufs=4))
        outp = ctx.enter_context(tc.tile_pool(name="outp", bufs=4))
        recp = ctx.enter_context(tc.tile_pool(name="recp", bufs=8))
        # PSUM budget (8 banks): pp_ss 2 slots x [128,1024]f32 = 4 banks
        # (shared tag for scores/proj/V/X^T-transposes), pp_po 2 x
        # [128,512]bf16 = 2 banks (output transposes), pp_av 2 x [65,512] = 2.
        pp_ss = ctx.enter_context(tc.tile_pool(name="pp_ss", bufs=2, space="PSUM"))
        pp_po = ctx.enter_context(tc.tile_pool(name="pp_po", bufs=2, space="PSUM"))
        pp_av = ctx.enter_context(tc.tile_pool(name="pp_av", bufs=2, space="PSUM"))

        rep_ctx = tc.For_i(0, reps, 1) if reps > 1 else nullcontext()
        with rep_ctx:
            ident = big.tile([P, P], F32)
            make_identity(nc, ident[:])
            identb = big.tile([P, P], BF16)
            nc.vector.tensor_copy(identb[:], ident[:])

            # ---- persistent SBUF tensors (all bf16) ----
            XT = big.tile([P, KT * S], BF16)       # X^T: col(kt, s) = kt*S + s
            WQs = big.tile([P, KT * HG], BF16)     # col(kt, j) = kt*HG + j
            WKs = big.tile([P, KT * HG], BF16)
            WVs = big.tile([P, KT * HG], BF16)
            QT = big.tile([P, 2 * S], BF16)        # col(jtile, m) = jtile*S + m
            KTt = big.tile([P, 2 * S], BF16)
            Vn = big.tile([P, ST * HPC * VW], BF16)  # col(st, h, e) = st*HPC*VW + h*VW + e

            # ---- load weights (one DMA per weight matrix), off SP's queue so
            # they never delay the X chunks that gate the first transposes ----
            for eng, Wt, w in (
                (nc.scalar, WQs, wq),
                (nc.gpsimd, WKs, wk),
                (nc.gpsimd, WVs, wv),
            ):
                eng.dma_start(
                    Wt[:].rearrange("p (k n) -> p k n", n=HG),
                    w[:].rearrange("(k p) n -> p k n", p=P),
                )

            # ones columns of Vn (col 64 of each head block)
            ones_ap = Vn[:].rearrange("p (s h e) -> p s h e", h=HPC, e=VW)[:, :, :, HD:VW]
            ones_stage = big.tile([P, ST * HPC], F32)
            nc.vector.memset(ones_stage[:], 1.0)
            nc.vector.tensor_copy(
                ones_ap,
                ones_stage[:].rearrange("p (s h e) -> p s h e", h=HPC, e=1),
            )

            # ---- prefix building blocks (X^T tiles, per-chunk projections).
            # All psum evacuations run on DVE: ACT must do nothing but exp.
            XT3 = XT[:].rearrange("p (k s) -> p k s", s=S)
            Vn4 = Vn[:].rearrange("p (s h e) -> p s h e", h=HPC, e=VW)

            def emit_x_group(sp):
                # DMA one 2-s-tile chunk of X and transpose it into XT
                xs = xst.tile([P, 2 * D], BF16)
                nc.sync.dma_start(
                    xs[:].rearrange("p (t d) -> p t d", d=D),
                    x[sp * 2 * P:(sp + 1) * 2 * P, :].rearrange("(t p) d -> p t d", p=P),
                )
                for tt in range(2):
                    st = sp * 2 + tt
                    for g in range(2):
                        pt = pp_ss.tile([P, 1024], BF16, tag="ps")
                        for j in range(4):
                            kt = g * 4 + j
                            nc.tensor.transpose(
                                pt[:, j * P:(j + 1) * P],
                                xs[:, tt * D + kt * P: tt * D + (kt + 1) * P],
                                identb[:],
                            )
                        dst = XT3[:, g * 4:(g + 1) * 4, st * P:(st + 1) * P]
                        src = pt[:, 0:512].rearrange("p (k s) -> p k s", s=P)
                        nc.vector.tensor_copy(dst, src)

            def emit_proj_qk_nn(Wt, Ot, j, nn):
                pt = pp_ss.tile([P, 1024], F32, tag="ps", name="pt")
                for kt in range(KT):
                    nc.tensor.matmul(
                        pt[:, 0:512],
                        Wt[:, kt * HG + j * P: kt * HG + (j + 1) * P],
                        XT[:, kt * S + nn * 512: kt * S + (nn + 1) * 512],
                        start=(kt == 0),
                        stop=(kt == KT - 1),
                    )
                nc.vector.tensor_copy(
                    Ot[:, j * S + nn * 512: j * S + (nn + 1) * 512],
                    pt[:, 0:512],
                )

            def emit_proj_qk(Wt, Ot, j):
                for nn in range(4):
                    emit_proj_qk_nn(Wt, Ot, j, nn)

            def emit_v_st(st):
                pt = pp_ss.tile([P, 1024], F32, tag="ps", name="pt")
                for kt in range(KT):
                    nc.tensor.matmul(
                        pt[:, 0:HG],
                        XT[:, kt * S + st * P: kt * S + (st + 1) * P],
                        WVs[:, kt * HG:(kt + 1) * HG],
                        start=(kt == 0),
                        stop=(kt == KT - 1),
                    )
                nc.vector.tensor_copy(
                    Vn4[:, st, :, 0:HD],
                    pt[:, 0:HG].rearrange("p (h e) -> p h e", e=HD),
                )

            def mm_scores(j, t, mc):
                # the two heads' score matmuls contract over disjoint
                # 64-row groups of the PE array -> concurrent execution
                qh = QT[:, j * S:(j + 1) * S]
                kh = KTt[:, j * S:(j + 1) * S]
                ps = pp_ss.tile([P, 2 * MC], F32, tag="ps", name="ps")
                nc.tensor.matmul(
                    ps[:, 0:MC],
                    kh[0:64, t * P:(t + 1) * P],
                    qh[0:64, mc * MC:(mc + 1) * MC],
                    start=True,
                    stop=True,
                )
                nc.tensor.matmul(
                    ps[:, MC:2 * MC],
                    kh[64:128, t * P:(t + 1) * P],
                    qh[64:128, mc * MC:(mc + 1) * MC],
                    start=True,
                    stop=True,
                )
                return ps

            class AttnChunk:
                """One (pair, m-chunk): av accumulators live across split
                emission so prefix work can interleave with the t-loop."""

                def __init__(self, j, mc):
                    self.j, self.mc = j, mc
                    self.av01 = [
                        pp_av.tile([VW, MC], F32, tag="av", name="av"),
                        pp_av.tile([VW, MC], F32, tag="av", name="av"),
                    ]
                    # software pipeline: MM_s(t+1) is emitted BEFORE
                    # exp(t)/MM_av(t) so the PE never sits behind the exp.
                    self.ps = mm_scores(j, 0, mc)

                def run(self, t_lo, t_hi):
                    for t in range(t_lo, t_hi):
                        ps_cur = self.ps
                        if t + 1 < ST:
                            self.ps = mm_scores(self.j, t + 1, self.mc)
                        ex = expp.tile([P, 2 * MC], BF16, tag="ex", name="ex")
                        nc.scalar.activation(
                            ex[:], ps_cur[:], EXP, scale=1.0 / np.sqrt(HD)
                        )
                        for hi in range(2):
                            h = 2 * self.j + hi
                            nc.tensor.matmul(
                                self.av01[hi][:],
                                Vn[:, t * HPC * VW + h * VW: t * HPC * VW + (h + 1) * VW],
                                ex[:, hi * MC:(hi + 1) * MC],
                                start=(t == 0),
                                stop=(t == ST - 1),
                            )

                def finish(self):
                    # evacuate, transpose back, normalize, store (one DMA per
                    # (head, m-chunk)). All 4 transposed 128-blocks land in
                    # ONE [128,512] psum tile so a single reciprocal + a
                    # single broadcast multiply normalize the whole chunk.
                    j, mc = self.j, self.mc
                    NMS = MC // P
                    for hi in range(2):
                        h = 2 * j + hi
                        osb = osbp.tile([VW, MC], BF16, tag="osb", name="osb")
                        nc.vector.tensor_copy(osb[:], self.av01[hi][:])
                        ot = outp.tile([P, NMS * HD], F32, tag="ot", name="ot")
                        po = pp_po.tile([P, 512], BF16, tag="po", name="po")
                        for ms in range(NMS):
                            nc.tensor.transpose(
                                po[:, ms * P:ms * P + VW],
                                osb[0:VW, ms * P:(ms + 1) * P],
                                identb[0:VW, 0:VW],
                            )
                        po3 = po[:].rearrange("p (ms c) -> p ms c", c=P)
                        rec = recp.tile([P, NMS], F32, tag="rec", name="rec")
                        nc.vector.reciprocal(rec[:], po3[:, :, HD])
                        nc.vector.tensor_tensor(
                            ot[:].rearrange("p (ms e) -> p ms e", e=HD),
                            po3[:, :, 0:HD],
                            rec[:].unsqueeze(2).broadcast_to([P, NMS, HD]),
                            MULT,
                        )
                        row0 = mc * MC
                        nc.sync.dma_start(
                            out[row0:row0 + MC, h * HD:(h + 1) * HD].rearrange(
                                "(t p) e -> p t e", p=P
                            ),
                            ot[:].rearrange("p (t e) -> p t e", e=HD),
                        )

            def emit_attn_chunk(j, mc, hooks=()):
                # hooks: [(t_split, fn), ...] -- fn() is emitted between
                # t-ranges so PE-only projection work lands inside this
                # chunk's ACT-bound slack instead of stalling the exp stream
                # at a chunk boundary.
                c = AttnChunk(j, mc)
                t0 = 0
                for ts, fn in hooks:
                    c.run(t0, ts)
                    fn()
                    t0 = ts
                c.run(t0, ST)
                c.finish()

            # ---- emission schedule ----
            # Prefix interleave: per nn-group, land the X chunks, then the
            # K/V/Q projections that depend only on them; attention chunk
            # (pair0, mc0) starts its t-loop as soon as the K^T/V tiles for
            # those t exist, so the ACT exp stream (the kernel bottleneck)
            # starts ~15-20us earlier than a phase-ordered emission.
            # nn=0: X s-tiles 0-3, K(0), V(0-3), Q(0)
            emit_x_group(0)
            emit_x_group(1)
            emit_proj_qk_nn(WKs, KTt, 0, 0)
            for st in range(0, 4):
                emit_v_st(st)
            emit_proj_qk_nn(WQs, QT, 0, 0)
            # nn=1: X s-tiles 4-7, K(1), V(4-7), Q(1), then attention t=0..3
            emit_x_group(2)
            emit_x_group(3)
            emit_proj_qk_nn(WKs, KTt, 0, 1)
            for st in range(4, 8):
                emit_v_st(st)
            emit_proj_qk_nn(WQs, QT, 0, 1)
            c0 = AttnChunk(0, 0)
            c0.run(0, 4)
            # nn=2: X s-tiles 8-11, K(2), attention t=4..7, V(8-11), Q(2)
            emit_x_group(4)
            emit_x_group(5)
            emit_proj_qk_nn(WKs, KTt, 0, 2)
            c0.run(4, 8)
            for st in range(8, 12):
                emit_v_st(st)
            emit_proj_qk_nn(WQs, QT, 0, 2)
            # nn=3: X s-tiles 12-15, K(3), attention t=8..11, V(12-15),
            # Q(3), attention t=12..15
            emit_x_group(6)
            emit_x_group(7)
            emit_proj_qk_nn(WKs, KTt, 0, 3)
            c0.run(8, 12)
            for st in range(12, 16):
                emit_v_st(st)
            emit_proj_qk_nn(WQs, QT, 0, 3)
            c0.run(12, ST)
            c0.finish()
            # Pair-1 projections are spread one nn-chunk at a time into the
            # remaining attention chunks' slack (each chunk is ACT-bound by
            # ~2.5us); Q1(0)/K1(3) land before AttnChunk(1,0) needs them.
            K1 = lambda nn: (lambda: emit_proj_qk_nn(WKs, KTt, 1, nn))
            Q1 = lambda nn: (lambda: emit_proj_qk_nn(WQs, QT, 1, nn))
            emit_attn_chunk(0, 1, [(6, K1(0))])
            emit_attn_chunk(0, 2, [(6, K1(1))])
            emit_attn_chunk(0, 3, [(4, K1(2)), (8, K1(3)), (12, Q1(0))])
            emit_attn_chunk(1, 0, [(6, Q1(1))])
            emit_attn_chunk(1, 1, [(6, Q1(2))])
            emit_attn_chunk(1, 2, [(6, Q1(3))])
            emit_attn_chunk(1, 3)

    nc.compile()
    return nc


_NC = None


def _get_nc():
    global _NC
    if _NC is None:
        _NC = build_nc()
    return _NC


def _shard_inputs(inputs, Wq, Wk, Wv):
    bf = ml_dtypes.bfloat16
    inputs = np.ascontiguousarray(np.asarray(inputs, dtype=np.float32)).astype(bf)
    Wq = np.asarray(Wq, dtype=np.float32).astype(bf)
    Wk = np.asarray(Wk, dtype=np.float32).astype(bf)
    Wv = np.asarray(Wv, dtype=np.float32).astype(bf)
    in_maps = []
    for c in range(NCORES):
        b, g = c // (NCORES // B), c % (NCORES // B)
        sl = slice(g * HG, (g + 1) * HG)
        in_maps.append(
            {
                "x": inputs[b],
                "wq": np.ascontiguousarray(Wq[:, sl]),
                "wk": np.ascontiguousarray(Wk[:, sl]),
                "wv": np.ascontiguousarray(Wv[:, sl]),
            }
        )
    return in_maps


def _gather(results):
    out = np.empty((B, S, H * HD), dtype=np.float32)
    for c in range(NCORES):
        b, g = c // (NCORES // B), c % (NCORES // B)
        out[b, :, g * HG:(g + 1) * HG] = results[c]["out"]
    return out


def kernel(inputs, Wq, Wk, Wv):
    nc = _get_nc()
    in_maps = _shard_inputs(inputs, Wq, Wk, Wv)
    res = run_bass_kernel_spmd(nc, in_maps, core_ids=list(range(NCORES)))
    return _gather(res.results)
